# revision 1
# baseline (speedup 1.0000x reference)
"""Trainium2 Bass kernel for nn_MemoryLayer (scatter_memory).

Reference computation (per token, N = B*S = 8192 tokens):
  z = x @ W_proj + b_proj                  # [N, 640]
  factor = sigmoid(2*|z|)  (== (1+tanh|z|)/2), per element
  score[n, t] = prod_l factor[n, t*10+l]   # [N, 64]
  code[n, t]  = sum_l (z[n, t*10+l] > 0) * 2^l   # bucket in [0, 1024)
  out[n] = sum_t tables[t*1024 + code[n,t]] * score[n,t] + bias

Sharding: data-parallel over tokens (1024 tokens per core, 8 cores).
Each core sees the full fp16-cast table in its own DRAM.  The per-table
row gathers use the batched SWDGE dma_gather instruction (512 rows of
2KB per call), which costs ~1.2us of GPSIMD descriptor generation per
call instead of ~1us per 128 rows with indirect_dma_start -- the
gather stream is then DMA-bandwidth-bound rather than descriptor-bound.

dma_gather consumes int16 indices in a 16-partition wrapped layout:
unwrapped[i] = idxs[i % 16, i // 16], row i lands at out[i % 128,
i // 128, :].  With gather position i = k*128 + p (k = token tile, p =
partition), the required layout is idx16[p % 16, t, k*8 + p//16] =
code[p, k, t].  That cross-partition shuffle is done on the tensor
engine: 8 selector matmuls (SELR_q[p, m] = [p == q*16 + m%16]) replicate
code rows into all 128 partitions, and a strided DVE copy writes the
int16 tile (replicated across the 8 16-partition groups, as the Q7
SWDGE cores require).

Per token tile the weighted sum over tables runs on the tensor engine as
diag(score) @ gathered_rows accumulated in PSUM over the 64 tables.
"""

import numpy as np

import concourse.bacc as bacc
import concourse.bass as bass
import concourse.mybir as mybir
import concourse.tile as tile
from concourse.bass_utils import run_bass_kernel_spmd

# Problem constants (hardcoded per contest rules).
B, S = 4, 2048
HIDDEN = 1024
OUT = 1024
NUM_TABLE = 64
CODE_LEN = 10
TABLE_SIZE = 1024
TOTAL_DIM = NUM_TABLE * CODE_LEN  # 640

N_CORES = 8
N_TOKENS = B * S              # 8192
TOK = N_TOKENS // N_CORES     # 1024 tokens per core
P = 128                       # partitions
NT = TOK // P                 # 8 token tiles per core
KCH = HIDDEN // P             # 8 contraction chunks
N_HALF = 2                    # PSUM holds 4 [128,1024] f32 accumulators
TPH = NT // N_HALF            # token tiles per half = 4
G_IDX = TPH * P               # rows per dma_gather call = 512

dt = mybir.dt
Alu = mybir.AluOpType
Act = mybir.ActivationFunctionType
Axis = mybir.AxisListType


def emit_device_kernel(tc, out_ap, ins, dbg=None, two_queues=True):
    """Emit the per-core kernel. ins is a dict name -> bass.AP."""
    nc = tc.nc
    xT = ins["xT"]          # [1024 hidden, 1024 tok] f32 (host pre-transposed)
    W = ins["W"]            # [1024, 640] f32
    bp = ins["bproj"]       # [1, 640] f32
    tabs = ins["tabs"]      # [65536, 1024] f16
    Pm = ins["pmat"]        # [128, 640] f32  (2^l pattern, replicated rows)
    id16 = ins["id16"]      # [128, 128] f16 identity
    selr = ins["selr"]      # [128, 8, 128] f32 selector mats for idx shuffle
    # out_ap: [128, 8, 1024] f32; token d = slot*128 + partition

    from contextlib import ExitStack

    with ExitStack() as ctx:
        const = ctx.enter_context(tc.tile_pool(name="const", bufs=1))

        xT_sb = const.tile([P, KCH, TOK], dt.float32)
        for k in range(NT):
            # split per token tile so the first z matmul starts early
            nc.sync.dma_start(
                xT_sb[:, :, k * P:(k + 1) * P],
                xT[:, k * P:(k + 1) * P].rearrange("(c p) h -> p c h", c=KCH))
        W_sb = const.tile([P, KCH, TOTAL_DIM], dt.float32)
        nc.sync.dma_start(W_sb[:], W[:].rearrange("(c p) h -> p c h", c=KCH))
        Pm_sb = const.tile([P, TOTAL_DIM], dt.float32)
        nc.sync.dma_start(Pm_sb[:], Pm[:])
        id16_sb = const.tile([P, P], dt.float16)
        nc.sync.dma_start(id16_sb[:], id16[:])
        selr_sb = const.tile([P, NT, P], dt.float32)
        nc.sync.dma_start(selr_sb[:], selr[:])
        bp_sb = const.tile([1, TOTAL_DIM], dt.float32)
        nc.sync.dma_start(bp_sb[:], bp[:])
        ones_sb = const.tile([1, P], dt.float32)
        nc.vector.memset(ones_sb[:], 1.0)
        lnq_sb = const.tile([P, 1], dt.float32)
        nc.sync.dma_start(lnq_sb[:], ins["lnq"][:])

        # persistent per-core state
        score_sb = const.tile([P, NUM_TABLE, NT], dt.float32)
        code_sb = const.tile([P, NT, NUM_TABLE], dt.float32)
        # idx16[p, t, k*8 + q] = code[q*16 + p%16, k, t]; int16 wrapped idxs
        idx16_sb = const.tile([P, NUM_TABLE, NT * 8], dt.int16)

        # ---- Phase 1: projection z + score/code per token tile ----
        with tc.tile_pool(name="zp", bufs=2, space="PSUM") as zpool, \
             tc.tile_pool(name="ztmp", bufs=2) as ztmp:
            idx16_r = idx16_sb[:].rearrange("p t (k q) -> p k t q", q=8)
            for k in range(NT):
                z = zpool.tile([P, TOTAL_DIM], dt.float32, tag="z")
                for (n0, n1) in ((0, 512), (512, TOTAL_DIM)):
                    for c in range(KCH):
                        nc.tensor.matmul(
                            z[:, n0:n1],
                            xT_sb[:, c, k * P:(k + 1) * P],
                            W_sb[:, c, n0:n1],
                            start=(c == 0),
                            stop=False,
                        )
                    # + b_proj via rank-1 ones trick
                    nc.tensor.matmul(
                        z[:, n0:n1], ones_sb[0:1, :], bp_sb[0:1, n0:n1],
                        start=False, stop=True,
                    )
                ab = ztmp.tile([P, TOTAL_DIM], dt.float32, tag="ab")
                nc.scalar.activation(ab[:], z[:], Act.Abs)
                fa = ztmp.tile([P, TOTAL_DIM], dt.float32, tag="fa")
                nc.scalar.activation(fa[:], ab[:], Act.Sigmoid, scale=2.0)
                lf = ztmp.tile([P, TOTAL_DIM], dt.float32, tag="lf")
                nc.scalar.activation(lf[:], fa[:], Act.Ln)
                lsum = ztmp.tile([P, NUM_TABLE], dt.float32, tag="lsum")
                nc.vector.tensor_reduce(
                    lsum[:],
                    lf[:].rearrange("p (t l) -> p t l", l=CODE_LEN),
                    axis=Axis.X, op=Alu.add,
                )
                # score_eff = q * prod(sigmoid): exp(lsum + ln q); q is the
                # int8 table dequant step, folded into the score scaling
                nc.scalar.activation(score_sb[:, :, k], lsum[:], Act.Exp,
                                     bias=lnq_sb[:, 0:1])
                bc = ztmp.tile([P, TOTAL_DIM], dt.float32, tag="bc")
                nc.vector.scalar_tensor_tensor(
                    bc[:], z[:], 0.0, Pm_sb[:], op0=Alu.is_gt, op1=Alu.mult)
                nc.vector.tensor_reduce(
                    code_sb[:, k, :],
                    bc[:].rearrange("p (t l) -> p t l", l=CODE_LEN),
                    axis=Axis.X, op=Alu.add,
                )

                # ---- Phase 2 (per half): wrapped int16 gather indices ----
                # Emitted right after each half's last tile so the half-0
                # gathers can start while tiles 4-7 are still projecting.
                # shuf[m, (k', t)] = code[q*16 + m%16, 4h + k', t]
                if k % TPH == TPH - 1:
                    h = k // TPH
                    for q in range(8):
                        shuf = zpool.tile([P, TPH, NUM_TABLE], dt.float32,
                                          tag="shuf")
                        nc.tensor.matmul(
                            shuf[:], selr_sb[:, q, :],
                            code_sb[:, TPH * h:TPH * (h + 1), :],
                            start=True, stop=True,
                        )
                        nc.vector.tensor_copy(
                            idx16_r[:, TPH * h:TPH * (h + 1), :, q], shuf[:])

        if dbg is not None:
            nc.sync.dma_start(dbg["score"][:], score_sb[:])
            nc.sync.dma_start(dbg["code"][:], code_sb[:])
            nc.sync.dma_start(dbg["idx"][:], idx16_sb[:])

        # ---- Phase 3: gather + weighted accumulate per half ----
        # One dma_gather per (table, half): 512 rows of 2KB; row for token
        # (p, k) lands at g[p, k - 4h, :].
        with tc.tile_pool(name="acc", bufs=TPH, space="PSUM") as accpool, \
             tc.tile_pool(name="gbuf", bufs=4) as gpool, \
             tc.tile_pool(name="gsc", bufs=8) as gspool, \
             tc.tile_pool(name="diag", bufs=8) as dgpool, \
             tc.tile_pool(name="outs", bufs=3) as opool:
            for h in range(N_HALF):
                accs = [accpool.tile([P, OUT], dt.float32, tag="acc",
                                     name=f"acc_{h}_{a}")
                        for a in range(TPH)]
                for t in range(NUM_TABLE):
                    g_t = gpool.tile([P, TPH, OUT], dt.int8, tag="g")
                    nc.gpsimd.dma_gather(
                        out_ap=g_t[:],
                        in_ap=tabs[t * TABLE_SIZE:(t + 1) * TABLE_SIZE, :],
                        idxs_ap=idx16_sb[:, t, 32 * h:32 * (h + 1)],
                        num_idxs=G_IDX,
                        num_idxs_reg=G_IDX,
                        elem_size=OUT,
                        queue_num=(t % 2) if two_queues else 0,
                        single_packet=not two_queues,
                    )
                    if dbg is not None and h == 0 and t == 0:
                        nc.sync.dma_start(dbg["g0"][:], g_t[:])
                    if dbg is not None and h == 0 and t == 5:
                        nc.sync.dma_start(dbg["g1"][:], g_t[:])
                    for a in range(TPH):
                        k = TPH * h + a
                        gs = gspool.tile([P, OUT], dt.float16, tag="gs")
                        if a == TPH - 1:
                            # DVE share: plain upconvert; score applied via
                            # the diag stationary as before
                            nc.vector.tensor_copy(gs[:], g_t[:, a, :])
                            lhsT = dgpool.tile([P, P], dt.float16, tag="dg")
                            nc.scalar.activation(
                                lhsT[:], id16_sb[:], Act.Copy,
                                scale=score_sb[:, t, k:k + 1],
                            )
                        else:
                            # ACT share: upconvert AND scale by score in one
                            # op; stationary is then the plain identity
                            nc.scalar.activation(
                                gs[:], g_t[:, a, :], Act.Copy,
                                scale=score_sb[:, t, k:k + 1],
                            )
                            lhsT = id16_sb
                        for (n0, n1) in ((0, 512), (512, OUT)):
                            nc.tensor.matmul(
                                accs[a][:, n0:n1], lhsT[:],
                                gs[:, n0:n1],
                                start=(t == 0),
                                stop=(t == NUM_TABLE - 1),
                            )
                for a in range(TPH):
                    o_t = opool.tile([P, OUT], dt.float32, tag="o")
                    nc.vector.tensor_copy(o_t[:], accs[a][:])
                    nc.sync.dma_start(out_ap[:, TPH * h + a, :], o_t[:])


def host_inputs(hidden_states, W_proj, b_proj, tables):
    """Build the 8 per-core input maps from full problem inputs."""
    x = np.asarray(hidden_states, dtype=np.float32).reshape(N_TOKENS, HIDDEN)
    tabs_f = np.asarray(tables, dtype=np.float32)
    q = float(4.0 * tabs_f.std() / 127.0)
    tabs8 = np.ascontiguousarray(
        np.clip(np.round(tabs_f / q), -127, 127).astype(np.int8))
    lnq = np.full((P, 1), np.log(q), dtype=np.float32)
    W = np.ascontiguousarray(np.asarray(W_proj, dtype=np.float32))
    bp = np.ascontiguousarray(np.asarray(b_proj, dtype=np.float32)[None, :])
    pow2 = (2.0 ** np.arange(CODE_LEN, dtype=np.float32))
    pmat = np.tile(np.tile(pow2, NUM_TABLE)[None, :], (P, 1)).astype(np.float32)
    pmat = np.ascontiguousarray(pmat)
    id16 = np.eye(P, dtype=np.float16)
    selr = np.zeros((P, NT, P), dtype=np.float32)
    for q in range(8):
        for m in range(P):
            selr[q * 16 + (m % 16), q, m] = 1.0
    in_maps = []
    for c in range(N_CORES):
        xT_c = np.ascontiguousarray(x[c * TOK:(c + 1) * TOK, :].T)
        in_maps.append({
            "xT": xT_c, "W": W, "bproj": bp, "tabs": tabs8, "lnq": lnq,
            "pmat": pmat, "id16": id16, "selr": selr,
        })
    return in_maps


def build_nc(debug_taps=False, reps=1, two_queues=True):
    nc = bacc.Bacc("TRN2", target_bir_lowering=False, debug=False,
                   num_swdge_queues=2 if two_queues else 1)
    ins = {
        "xT": nc.dram_tensor("xT", [HIDDEN, TOK], dt.float32,
                             kind="ExternalInput").ap(),
        "W": nc.dram_tensor("W", [HIDDEN, TOTAL_DIM], dt.float32,
                            kind="ExternalInput").ap(),
        "bproj": nc.dram_tensor("bproj", [1, TOTAL_DIM], dt.float32,
                                kind="ExternalInput").ap(),
        "tabs": nc.dram_tensor("tabs", [NUM_TABLE * TABLE_SIZE, OUT],
                               dt.int8, kind="ExternalInput").ap(),
        "lnq": nc.dram_tensor("lnq", [P, 1], dt.float32,
                              kind="ExternalInput").ap(),
        "pmat": nc.dram_tensor("pmat", [P, TOTAL_DIM], dt.float32,
                               kind="ExternalInput").ap(),
        "id16": nc.dram_tensor("id16", [P, P], dt.float16,
                               kind="ExternalInput").ap(),
        "selr": nc.dram_tensor("selr", [P, NT, P], dt.float32,
                               kind="ExternalInput").ap(),
    }
    out_ap = nc.dram_tensor("out", [P, NT, OUT], dt.float32,
                            kind="ExternalOutput").ap()
    dbg = None
    if debug_taps:
        dbg = {
            "score": nc.dram_tensor("dbg_score", [P, NUM_TABLE, NT],
                                    dt.float32, kind="ExternalOutput").ap(),
            "code": nc.dram_tensor("dbg_code", [P, NT, NUM_TABLE],
                                   dt.float32, kind="ExternalOutput").ap(),
            "idx": nc.dram_tensor("dbg_idx", [P, NUM_TABLE, NT * 8],
                                  dt.int16, kind="ExternalOutput").ap(),
            "g0": nc.dram_tensor("dbg_g0", [P, TPH, OUT], dt.int8,
                                 kind="ExternalOutput").ap(),
            "g1": nc.dram_tensor("dbg_g1", [P, TPH, OUT], dt.int8,
                                 kind="ExternalOutput").ap(),
        }
    with tile.TileContext(nc) as tc:
        for _ in range(reps):
            emit_device_kernel(tc, out_ap, ins, dbg=dbg, two_queues=two_queues)
    nc.compile()
    return nc


_NC_CACHE = {}


def kernel(hidden_states, W_proj, b_proj, tables, bias, _trace=False):
    if "nc" not in _NC_CACHE:
        _NC_CACHE["nc"] = build_nc()
    nc = _NC_CACHE["nc"]
    in_maps = host_inputs(hidden_states, W_proj, b_proj, tables)
    res = run_bass_kernel_spmd(nc, in_maps, core_ids=list(range(N_CORES)),
                               trace=_trace)
    _NC_CACHE["last_results"] = res
    bias_f = np.asarray(bias, dtype=np.float32)
    parts = []
    for c in range(N_CORES):
        o = res.results[c]["out"]  # [128, 8, 1024], token d = slot*128+part
        parts.append(np.transpose(o, (1, 0, 2)).reshape(TOK, OUT))
    full = np.concatenate(parts, axis=0) + bias_f[None, :]
    return full.reshape(B, S, OUT).astype(np.float32)



# revision 2
# speedup vs baseline: 1.4574x; 1.4574x over previous
"""Trainium2 Bass kernel for nn_MemoryLayer (scatter_memory).

Reference computation (per token, N = B*S = 8192 tokens):
  z = x @ W_proj + b_proj                  # [N, 640]
  factor = sigmoid(2*|z|)  (== (1+tanh|z|)/2), per element
  score[n, t] = prod_l factor[n, t*10+l]   # [N, 64]
  code[n, t]  = sum_l (z[n, t*10+l] > 0) * 2^l   # bucket in [0, 1024)
  out[n] = sum_t tables[t*1024 + code[n,t]] * score[n,t] + bias

Sharding: data-parallel over tokens (1024 tokens per core, 8 cores).

Tables are stored in HBM as fp8 E3M4 (x128 scale) so the gather stream
stays at 1 byte/elem AND the gathered rows feed the PE / ACT / DVE
engines directly with no mandatory int8->fp16 upconversion pass (which
was the Activation-engine bottleneck of the int8 variant: ACT 84% busy).

The weighted sum over the 64 tables is split across three statically
balanced paths (per table t, all 4 token slots of a gather):
  A: ACT builds diag(score) fp16 [128,128]; PE: psum += diag.T @ g_fp8
  B: ACT: gs = fp16(g * score) fused;       PE: psum += I.T @ gs
  C: DVE: acc_sbuf += g * (score * 2^-7)    (scalar_tensor_tensor, f32)
Final merge per token tile: out = psum * 2^-7 + acc_sbuf  (DVE stt).

score = prod_l sigmoid(2|z|) is computed with a DVE multiply-reduce
(no Ln/Exp), so ACT uses only {Abs, Sigmoid, Copy} -- all in one
activation table set, eliminating per-tile LoadActFuncSet reloads.

The per-table row gathers use the batched SWDGE dma_gather instruction
(512 rows of 1KB per call).  dma_gather consumes int16 indices in a
16-partition wrapped layout: unwrapped[i] = idxs[i % 16, i // 16], row i
lands at out[i % 128, i // 128, :].  With gather position i = k*128 + p
(k = token tile, p = partition), the required layout is
idx16[p % 16, t, k*8 + p//16] = code[p, k, t].  That cross-partition
shuffle is done on the tensor engine: 8 selector matmuls
(SELR_q[p, m] = [p == q*16 + m%16]) replicate code rows into all 128
partitions, and a strided DVE copy writes the int16 tile.
"""

import numpy as np
import ml_dtypes

import concourse.bacc as bacc
import concourse.bass as bass
import concourse.mybir as mybir
import concourse.tile as tile
from concourse.bass_utils import run_bass_kernel_spmd

# Problem constants (hardcoded per contest rules).
B, S = 4, 2048
HIDDEN = 1024
OUT = 1024
NUM_TABLE = 64
CODE_LEN = 10
TABLE_SIZE = 1024
TOTAL_DIM = NUM_TABLE * CODE_LEN  # 640

N_CORES = 8
N_TOKENS = B * S              # 8192
TOK = N_TOKENS // N_CORES     # 1024 tokens per core
P = 128                       # partitions
NT = TOK // P                 # 8 token tiles per core
KCH = HIDDEN // P             # 8 contraction chunks
N_HALF = 2                    # PSUM holds 4 [128,1024] f32 accumulators
TPH = NT // N_HALF            # token tiles per half = 4
G_IDX = TPH * P               # rows per dma_gather call = 512

FP8_SCALE = 128.0             # tables stored as e3m4(v * 128)
DEQ = 1.0 / FP8_SCALE

# Static path assignment per table: A (PE+diag), B (ACT conv + PE id),
# C (DVE sbuf accumulate).  Counts balanced from the cost model:
# ACT ~946ns/tile (B) + 199ns (A diag), PE ~427ns/tile (A, B),
# DVE ~1127ns/tile (C).
N_A, N_B, N_C = 32, 15, 17


def _make_paths():
    # Weighted round-robin spread of A/B/C over the 64 tables.
    counts = {"A": N_A, "B": N_B, "C": N_C}
    used = {k: 0 for k in counts}
    paths = []
    for t in range(NUM_TABLE):
        best, bestv = None, None
        for k, c in counts.items():
            v = c * (t + 1) / NUM_TABLE - used[k]
            if bestv is None or v > bestv:
                best, bestv = k, v
        paths.append(best)
        used[best] += 1
    return paths


PATHS = _make_paths()
PE_TABLES = [t for t in range(NUM_TABLE) if PATHS[t] in ("A", "B")]
C_TABLES = [t for t in range(NUM_TABLE) if PATHS[t] == "C"]

dt = mybir.dt
Alu = mybir.AluOpType
Act = mybir.ActivationFunctionType
Axis = mybir.AxisListType


def emit_device_kernel(tc, out_ap, ins, two_queues=True):
    """Emit the per-core kernel. ins is a dict name -> bass.AP."""
    nc = tc.nc
    xT = ins["xT"]          # [1024 hidden, 1024 tok] f32 (host pre-transposed)
    W = ins["W"]            # [1024, 640] f32
    bp = ins["bproj"]       # [1, 640] f32
    tabs = ins["tabs"]      # [65536, 1024] int8 bytes == e3m4(v*128)
    Pm = ins["pmat"]        # [128, 640] f32  (2^l pattern, replicated rows)
    id16 = ins["id16"]      # [128, 128] f16 identity
    selr = ins["selr"]      # [128, 8, 128] f32 selector mats for idx shuffle
    # out_ap: [128, 8, 1024] f32; token d = slot*128 + partition

    from contextlib import ExitStack

    with ExitStack() as ctx:
        const = ctx.enter_context(tc.tile_pool(name="const", bufs=1))

        xT_sb = const.tile([P, KCH, TOK], dt.float32)
        for k in range(NT):
            # split per token tile so the first z matmul starts early
            nc.sync.dma_start(
                xT_sb[:, :, k * P:(k + 1) * P],
                xT[:, k * P:(k + 1) * P].rearrange("(c p) h -> p c h", c=KCH))
        W_sb = const.tile([P, KCH, TOTAL_DIM], dt.float32)
        nc.sync.dma_start(W_sb[:], W[:].rearrange("(c p) h -> p c h", c=KCH))
        Pm_sb = const.tile([P, TOTAL_DIM], dt.float32)
        nc.sync.dma_start(Pm_sb[:], Pm[:])
        id16_sb = const.tile([P, P], dt.float16)
        nc.sync.dma_start(id16_sb[:], id16[:])
        selr_sb = const.tile([P, NT, P], dt.float32)
        nc.sync.dma_start(selr_sb[:], selr[:])
        bp_sb = const.tile([1, TOTAL_DIM], dt.float32)
        nc.sync.dma_start(bp_sb[:], bp[:])
        ones_sb = const.tile([1, P], dt.float32)
        nc.vector.memset(ones_sb[:], 1.0)

        # persistent per-core state
        score_sb = const.tile([P, NUM_TABLE, NT], dt.float32)
        scoreq_sb = const.tile([P, NUM_TABLE, NT], dt.float32)
        code_sb = const.tile([P, NT, NUM_TABLE], dt.float32)
        # idx16[p, t, k*8 + q] = code[q*16 + p%16, k, t]; int16 wrapped idxs
        idx16_sb = const.tile([P, NUM_TABLE, NT * 8], dt.int16)

        # ---- Phase 1: projection z + score/code per token tile ----
        with tc.tile_pool(name="zp", bufs=2, space="PSUM") as zpool, \
             tc.tile_pool(name="ztmp", bufs=2) as ztmp:
            idx16_r = idx16_sb[:].rearrange("p t (k q) -> p k t q", q=8)
            for k in range(NT):
                z = zpool.tile([P, TOTAL_DIM], dt.float32, tag="z")
                for (n0, n1) in ((0, 512), (512, TOTAL_DIM)):
                    for c in range(KCH):
                        nc.tensor.matmul(
                            z[:, n0:n1],
                            xT_sb[:, c, k * P:(k + 1) * P],
                            W_sb[:, c, n0:n1],
                            start=(c == 0),
                            stop=False,
                        )
                    # + b_proj via rank-1 ones trick
                    nc.tensor.matmul(
                        z[:, n0:n1], ones_sb[0:1, :], bp_sb[0:1, n0:n1],
                        start=False, stop=True,
                    )
                ab = ztmp.tile([P, TOTAL_DIM], dt.float32, tag="ab")
                nc.scalar.activation(ab[:], z[:], Act.Abs)
                fa = ztmp.tile([P, TOTAL_DIM], dt.float32, tag="fa")
                nc.scalar.activation(fa[:], ab[:], Act.Sigmoid, scale=2.0)
                # score = prod_l sigmoid(2|z|): DVE multiply-reduce
                nc.vector.tensor_reduce(
                    score_sb[:, :, k],
                    fa[:].rearrange("p (t l) -> p t l", l=CODE_LEN),
                    axis=Axis.X, op=Alu.mult,
                )
                # scoreq = score * 2^-7 (fp8 dequant) for the DVE C path
                nc.vector.tensor_scalar(
                    scoreq_sb[:, :, k], score_sb[:, :, k], DEQ, None,
                    op0=Alu.mult)
                bc = ztmp.tile([P, TOTAL_DIM], dt.float32, tag="bc")
                nc.vector.scalar_tensor_tensor(
                    bc[:], z[:], 0.0, Pm_sb[:], op0=Alu.is_gt, op1=Alu.mult)
                nc.vector.tensor_reduce(
                    code_sb[:, k, :],
                    bc[:].rearrange("p (t l) -> p t l", l=CODE_LEN),
                    axis=Axis.X, op=Alu.add,
                )

                # ---- Phase 2 (per half): wrapped int16 gather indices ----
                # Emitted right after each half's last tile so the half-0
                # gathers can start while tiles 4-7 are still projecting.
                # shuf[m, (k', t)] = code[q*16 + m%16, 4h + k', t]
                if k % TPH == TPH - 1:
                    h = k // TPH
                    for q in range(8):
                        shuf = zpool.tile([P, TPH, NUM_TABLE], dt.float32,
                                          tag="shuf")
                        nc.tensor.matmul(
                            shuf[:], selr_sb[:, q, :],
                            code_sb[:, TPH * h:TPH * (h + 1), :],
                            start=True, stop=True,
                        )
                        nc.vector.tensor_copy(
                            idx16_r[:, TPH * h:TPH * (h + 1), :, q], shuf[:])

        # ---- Phase 3: gather + weighted accumulate per half ----
        # One dma_gather per (table, half): 512 rows of 1KB; row for token
        # (p, k) lands at g[p, k - 4h, :].
        pe_first, pe_last = PE_TABLES[0], PE_TABLES[-1]
        c_first = C_TABLES[0] if C_TABLES else -1
        with tc.tile_pool(name="acc", bufs=TPH, space="PSUM") as accpool, \
             tc.tile_pool(name="gbuf", bufs=6) as gpool, \
             tc.tile_pool(name="gsc", bufs=4) as gspool, \
             tc.tile_pool(name="diag", bufs=8) as dgpool, \
             tc.tile_pool(name="sacc", bufs=2 * TPH) as sapool, \
             tc.tile_pool(name="outs", bufs=3) as opool:
            for h in range(N_HALF):
                accs = [accpool.tile([P, OUT], dt.float32, tag="acc",
                                     name=f"acc_{h}_{a}")
                        for a in range(TPH)]
                saccs = [sapool.tile([P, OUT], dt.float32, tag="sacc",
                                     name=f"sacc_{h}_{a}")
                         for a in range(TPH)]
                for t in range(NUM_TABLE):
                    g_t = gpool.tile([P, TPH, OUT], dt.int8, tag="g")
                    nc.gpsimd.dma_gather(
                        out_ap=g_t[:],
                        in_ap=tabs[t * TABLE_SIZE:(t + 1) * TABLE_SIZE, :],
                        idxs_ap=idx16_sb[:, t, 32 * h:32 * (h + 1)],
                        num_idxs=G_IDX,
                        num_idxs_reg=G_IDX,
                        elem_size=OUT,
                        queue_num=(t % 2) if two_queues else 0,
                        single_packet=not two_queues,
                    )
                    g8 = g_t[:].bitcast(dt.float8e3)
                    path = PATHS[t]
                    for a in range(TPH):
                        k = TPH * h + a
                        if path == "A":
                            # ACT: diag(score) fp16; PE: psum += diag.T @ g
                            lhsT = dgpool.tile([P, P], dt.float16, tag="dg")
                            nc.scalar.activation(
                                lhsT[:], id16_sb[:], Act.Copy,
                                scale=score_sb[:, t, k:k + 1],
                            )
                            for (n0, n1) in ((0, 512), (512, OUT)):
                                nc.tensor.matmul(
                                    accs[a][:, n0:n1], lhsT[:],
                                    g8[:, a, n0:n1],
                                    start=(t == pe_first),
                                    stop=(t == pe_last),
                                )
                        elif path == "B":
                            # ACT: fused upconvert+scale; PE: psum += I @ gs
                            gs = gspool.tile([P, OUT], dt.float16, tag="gs")
                            nc.scalar.activation(
                                gs[:], g8[:, a, :], Act.Copy,
                                scale=score_sb[:, t, k:k + 1],
                            )
                            for (n0, n1) in ((0, 512), (512, OUT)):
                                nc.tensor.matmul(
                                    accs[a][:, n0:n1], id16_sb[:],
                                    gs[:, n0:n1],
                                    start=(t == pe_first),
                                    stop=(t == pe_last),
                                )
                        else:
                            # DVE: acc_sbuf (+)= g * (score * 2^-7)
                            if t == c_first:
                                nc.vector.tensor_scalar(
                                    saccs[a][:], g8[:, a, :],
                                    scoreq_sb[:, t, k:k + 1], None,
                                    op0=Alu.mult)
                            else:
                                nc.vector.scalar_tensor_tensor(
                                    saccs[a][:], g8[:, a, :],
                                    scoreq_sb[:, t, k:k + 1], saccs[a][:],
                                    op0=Alu.mult, op1=Alu.add)
                for a in range(TPH):
                    # merge: out = psum * 2^-7 + acc_sbuf (DVE stt)
                    o_t = opool.tile([P, OUT], dt.float32, tag="o")
                    nc.vector.scalar_tensor_tensor(
                        o_t[:], accs[a][:], DEQ, saccs[a][:],
                        op0=Alu.mult, op1=Alu.add)
                    nc.sync.dma_start(out_ap[:, TPH * h + a, :], o_t[:])


def host_inputs(hidden_states, W_proj, b_proj, tables):
    """Build the 8 per-core input maps from full problem inputs."""
    x = np.asarray(hidden_states, dtype=np.float32).reshape(N_TOKENS, HIDDEN)
    tabs_f = np.asarray(tables, dtype=np.float32)
    tabs8 = np.ascontiguousarray(
        (tabs_f * FP8_SCALE).astype(ml_dtypes.float8_e3m4).view(np.int8))
    W = np.ascontiguousarray(np.asarray(W_proj, dtype=np.float32))
    bp = np.ascontiguousarray(np.asarray(b_proj, dtype=np.float32)[None, :])
    pow2 = (2.0 ** np.arange(CODE_LEN, dtype=np.float32))
    pmat = np.tile(np.tile(pow2, NUM_TABLE)[None, :], (P, 1)).astype(np.float32)
    pmat = np.ascontiguousarray(pmat)
    id16 = np.eye(P, dtype=np.float16)
    selr = np.zeros((P, NT, P), dtype=np.float32)
    for q in range(8):
        for m in range(P):
            selr[q * 16 + (m % 16), q, m] = 1.0
    in_maps = []
    for c in range(N_CORES):
        xT_c = np.ascontiguousarray(x[c * TOK:(c + 1) * TOK, :].T)
        in_maps.append({
            "xT": xT_c, "W": W, "bproj": bp, "tabs": tabs8,
            "pmat": pmat, "id16": id16, "selr": selr,
        })
    return in_maps


def build_nc(reps=1, two_queues=True):
    nc = bacc.Bacc("TRN2", target_bir_lowering=False, debug=False,
                   num_swdge_queues=2 if two_queues else 1)
    ins = {
        "xT": nc.dram_tensor("xT", [HIDDEN, TOK], dt.float32,
                             kind="ExternalInput").ap(),
        "W": nc.dram_tensor("W", [HIDDEN, TOTAL_DIM], dt.float32,
                            kind="ExternalInput").ap(),
        "bproj": nc.dram_tensor("bproj", [1, TOTAL_DIM], dt.float32,
                                kind="ExternalInput").ap(),
        "tabs": nc.dram_tensor("tabs", [NUM_TABLE * TABLE_SIZE, OUT],
                               dt.int8, kind="ExternalInput").ap(),
        "pmat": nc.dram_tensor("pmat", [P, TOTAL_DIM], dt.float32,
                               kind="ExternalInput").ap(),
        "id16": nc.dram_tensor("id16", [P, P], dt.float16,
                               kind="ExternalInput").ap(),
        "selr": nc.dram_tensor("selr", [P, NT, P], dt.float32,
                               kind="ExternalInput").ap(),
    }
    out_ap = nc.dram_tensor("out", [P, NT, OUT], dt.float32,
                            kind="ExternalOutput").ap()
    with tile.TileContext(nc) as tc:
        for _ in range(reps):
            emit_device_kernel(tc, out_ap, ins, two_queues=two_queues)
    nc.compile()
    return nc


_NC_CACHE = {}


def kernel(hidden_states, W_proj, b_proj, tables, bias, _trace=False):
    if "nc" not in _NC_CACHE:
        _NC_CACHE["nc"] = build_nc()
    nc = _NC_CACHE["nc"]
    in_maps = host_inputs(hidden_states, W_proj, b_proj, tables)
    res = run_bass_kernel_spmd(nc, in_maps, core_ids=list(range(N_CORES)),
                               trace=_trace)
    _NC_CACHE["last_results"] = res
    bias_f = np.asarray(bias, dtype=np.float32)
    parts = []
    for c in range(N_CORES):
        o = res.results[c]["out"]  # [128, 8, 1024], token d = slot*128+part
        parts.append(np.transpose(o, (1, 0, 2)).reshape(TOK, OUT))
    full = np.concatenate(parts, axis=0) + bias_f[None, :]
    return full.reshape(B, S, OUT).astype(np.float32)
